# revision 2
# baseline (speedup 1.0000x reference)
"""Data-parallel Trainium2 kernel for nn_Actor_89842125897993.

Shards the batch dim B=4096 across 8 NeuronCores (512 batches/core).
All params are replicated. The 20-step scan runs per-shard; the only
cross-core coupling is the training-mode BatchNorm statistics, which
are computed with explicit psum allreduces (6 floats per BN) so they
match the full-batch reference semantics exactly.

Self-contained: hardcodes all shapes from the problem spec.
"""
import numpy as np
import jax
import jax.numpy as jnp
from jax.sharding import Mesh, PartitionSpec as P, NamedSharding
from functools import partial

B, N, T = 4096, 16, 20
S, D = 3, 3
H, A, PP = 128, 8, 8
EPS = 1e-5
M = 8  # cores

_cache = {}


def _build():
    devs = jax.devices()[:M]
    mesh = Mesh(np.array(devs), ("x",))

    def bn_global(x, w, b):
        # x: [Bl, N, C]; global (full-batch) mean/var via psum, matching
        # the reference's two-pass biased variance.
        m = jax.lax.psum(jnp.sum(x, axis=(0, 1)), "x") / (B * N)
        v = jax.lax.psum(jnp.sum((x - m) ** 2, axis=(0, 1)), "x") / (B * N)
        return (x - m) * jax.lax.rsqrt(v + EPS) * w + b

    def shard_fn(static, dynamic, sbn_w, sbn_b, dbn_w, dbn_b,
                 Ws, bs, Wd, bd, Wa, ba, Wp, bp):
        dt = static.dtype

        def step(dyn_t, static_t):
            s_h = bn_global(static_t, sbn_w, sbn_b) @ Ws + bs
            d_h = bn_global(dyn_t, dbn_w, dbn_b) @ Wd + bd
            state = jnp.concatenate([s_h, d_h], axis=2)
            actions = state @ Wa + ba
            aparams = state @ Wp + bp
            quant_probs = jax.nn.softmax(actions, axis=2)
            # argmax via single-operand reduces (variadic reduce unsupported):
            # first index attaining the max, matching jnp.argmax semantics.
            maxv = jnp.max(actions, axis=2, keepdims=True)
            idxs = jnp.arange(A, dtype=jnp.int32)
            eq = actions >= maxv
            ptr_quant = jnp.min(jnp.where(eq, idxs, A), axis=2)
            onehot = (idxs == ptr_quant[..., None]).astype(dt)
            bdw = jnp.sum(aparams * onehot, axis=2)
            bdw = jax.nn.softmax(bdw, axis=1)
            ptr_q8 = ptr_quant + 8
            d2 = dyn_t[:, :, 1] + 0.05 * static_t[:, :, 2]
            d2 = jnp.sqrt((d2 - 500.0) ** 2 + 100.0)
            rate = bdw * 10.0 * jnp.log2(1.0 + 1e7 * static_t[:, :, 0] / (d2 * d2))
            d0 = jnp.max(0.05 + ptr_q8.astype(dt) / 32.0 / rate, axis=1, keepdims=True)
            d1 = d0 * static_t[:, :, 2] + dyn_t[:, :, 1]
            new_dyn = jnp.stack([jnp.broadcast_to(d0, d2.shape), d1, d2], axis=2)
            wprobs = jnp.full(quant_probs.shape[:2] + (2,), 0.5, dtype=dt)
            ptr_sel = jnp.zeros(quant_probs.shape[:2], dtype=jnp.int32)
            return new_dyn, (ptr_sel, ptr_q8, bdw, wprobs, quant_probs, aparams)

        static_steps = jnp.moveaxis(static, 3, 0)
        dyn0 = dynamic[:, :, :, 0]
        _, (ptr_sel, ptr_q8, bdw, wprobs, quant_probs, aparams) = jax.lax.scan(
            step, dyn0, static_steps)

        tl = lambda x: jnp.moveaxis(x, 0, -1)
        action = jnp.concatenate([
            tl(ptr_sel).astype(dt)[:, :, None, :],
            tl(ptr_q8).astype(dt)[:, :, None, :],
            tl(bdw)[:, :, None, :]], axis=2)
        action_logp = jnp.concatenate(
            [tl(wprobs), tl(quant_probs), tl(aparams)], axis=2)
        return action, action_logp

    shard_spec = P("x")
    rep = P()
    in_specs = (shard_spec, shard_spec) + (rep,) * 12
    out_specs = (shard_spec, shard_spec)

    fn = jax.jit(jax.shard_map(shard_fn, mesh=mesh,
                               in_specs=in_specs, out_specs=out_specs))

    shardings = tuple(NamedSharding(mesh, s) for s in in_specs)
    return mesh, fn, shardings


def kernel(**inputs):
    if "fn" not in _cache:
        _cache["mesh"], _cache["fn"], _cache["shardings"] = _build()
    fn, shardings = _cache["fn"], _cache["shardings"]

    order = ["static", "dynamic", "sbn_w", "sbn_b", "dbn_w", "dbn_b",
             "Ws", "bs", "Wd", "bd", "Wa", "ba", "Wp", "bp"]
    args = [jax.device_put(np.asarray(inputs[k]), sh)
            for k, sh in zip(order, shardings)]
    action, action_logp = fn(*args)
    return (np.asarray(jax.device_get(action)),
            np.asarray(jax.device_get(action_logp)))


# revision 4
# speedup vs baseline: 25.2766x; 25.2766x over previous
"""Data-parallel Trainium2 kernel for nn_Actor_89842125897993.

Shards the batch dim B=4096 across 8 NeuronCores (512 batches/core).
All params are replicated. The 20-step scan runs per-shard; the only
cross-core coupling is the training-mode BatchNorm statistics, which
are computed with explicit psum allreduces (6 floats per BN) so they
match the full-batch reference semantics exactly.

Self-contained: hardcodes all shapes from the problem spec.
"""
import numpy as np
import jax
import jax.numpy as jnp
from jax.sharding import Mesh, PartitionSpec as P, NamedSharding
from functools import partial

B, N, T = 4096, 16, 20
S, D = 3, 3
H, A, PP = 128, 8, 8
EPS = 1e-5
M = 8  # cores

_cache = {}


def _build():
    devs = jax.devices()[:M]
    mesh = Mesh(np.array(devs), ("x",))

    def bn_global(x, w, b):
        # x: [Bl, N, C]; global (full-batch) mean/var via psum, matching
        # the reference's two-pass biased variance.
        m = jax.lax.psum(jnp.sum(x, axis=(0, 1)), "x") / (B * N)
        v = jax.lax.psum(jnp.sum((x - m) ** 2, axis=(0, 1)), "x") / (B * N)
        return (x - m) * jax.lax.rsqrt(v + EPS) * w + b

    def shard_fn(static, dynamic, sbn_w, sbn_b, dbn_w, dbn_b,
                 Ws, bs, Wd, bd, Wa, ba, Wp, bp):
        dt = static.dtype

        # The network is linear up to the softmax/argmax, so fold
        # BN-affine + Ws/Wd + Wa/Wp into per-step [3,16] weights:
        #   out = static_t @ (s_scale*Wsx) + dyn_t @ (d_scale*Wdx) + const
        Wsx = Ws @ jnp.concatenate([Wa[:H], Wp[:H]], axis=1)        # [3,16]
        Wdx = Wd @ jnp.concatenate([Wa[H:], Wp[H:]], axis=1)        # [3,16]
        cx = (bs @ jnp.concatenate([Wa[:H], Wp[:H]], axis=1)
              + bd @ jnp.concatenate([Wa[H:], Wp[H:]], axis=1)
              + jnp.concatenate([ba, bp]))                          # [16]

        # static BN stats for all T steps, once (global over full batch)
        sm = jax.lax.psum(jnp.sum(static, axis=(0, 1)), "x") / (B * N)   # [3,T]
        sv = jax.lax.psum(jnp.sum((static - sm) ** 2, axis=(0, 1)), "x") / (B * N)
        s_scale = sbn_w[:, None] * jax.lax.rsqrt(sv + EPS)               # [3,T]
        s_shift = sbn_b[:, None] - sm * s_scale                          # [3,T]

        def step(dyn_t, xs):
            static_t, s_scale_t, s_shift_t = xs           # [Bl,N,3], [3], [3]
            dm = jax.lax.psum(jnp.sum(dyn_t, axis=(0, 1)), "x") / (B * N)
            dv = jax.lax.psum(jnp.sum((dyn_t - dm) ** 2, axis=(0, 1)), "x") / (B * N)
            d_scale = dbn_w * jax.lax.rsqrt(dv + EPS)
            d_shift = dbn_b - dm * d_scale
            Ws_eff = s_scale_t[:, None] * Wsx             # [3,16]
            Wd_eff = d_scale[:, None] * Wdx               # [3,16]
            c_eff = cx + s_shift_t @ Wsx + d_shift @ Wdx  # [16]
            out16 = static_t @ Ws_eff + dyn_t @ Wd_eff + c_eff   # [Bl,N,16]
            actions = out16[:, :, :A]
            aparams = out16[:, :, A:]
            quant_probs = jax.nn.softmax(actions, axis=2)
            # argmax via single-operand reduces (variadic reduce unsupported):
            # first index attaining the max, matching jnp.argmax semantics.
            maxv = jnp.max(actions, axis=2, keepdims=True)
            idxs = jnp.arange(A, dtype=jnp.int32)
            eq = actions >= maxv
            ptr_quant = jnp.min(jnp.where(eq, idxs, A), axis=2)
            onehot = (idxs == ptr_quant[..., None]).astype(dt)
            bdw = jnp.sum(aparams * onehot, axis=2)
            bdw = jax.nn.softmax(bdw, axis=1)
            ptr_q8 = ptr_quant + 8
            d2 = dyn_t[:, :, 1] + 0.05 * static_t[:, :, 2]
            d2 = jnp.sqrt((d2 - 500.0) ** 2 + 100.0)
            rate = bdw * 10.0 * jnp.log2(1.0 + 1e7 * static_t[:, :, 0] / (d2 * d2))
            d0 = jnp.max(0.05 + ptr_q8.astype(dt) / 32.0 / rate, axis=1, keepdims=True)
            d1 = d0 * static_t[:, :, 2] + dyn_t[:, :, 1]
            new_dyn = jnp.stack([jnp.broadcast_to(d0, d2.shape), d1, d2], axis=2)
            wprobs = jnp.full(quant_probs.shape[:2] + (2,), 0.5, dtype=dt)
            ptr_sel = jnp.zeros(quant_probs.shape[:2], dtype=jnp.int32)
            return new_dyn, (ptr_sel, ptr_q8, bdw, wprobs, quant_probs, aparams)

        static_steps = jnp.moveaxis(static, 3, 0)
        dyn0 = dynamic[:, :, :, 0]
        _, (ptr_sel, ptr_q8, bdw, wprobs, quant_probs, aparams) = jax.lax.scan(
            step, dyn0, (static_steps, s_scale.T, s_shift.T))

        tl = lambda x: jnp.moveaxis(x, 0, -1)
        action = jnp.concatenate([
            tl(ptr_sel).astype(dt)[:, :, None, :],
            tl(ptr_q8).astype(dt)[:, :, None, :],
            tl(bdw)[:, :, None, :]], axis=2)
        action_logp = jnp.concatenate(
            [tl(wprobs), tl(quant_probs), tl(aparams)], axis=2)
        return action, action_logp

    shard_spec = P("x")
    rep = P()
    in_specs = (shard_spec, shard_spec) + (rep,) * 12
    out_specs = (shard_spec, shard_spec)

    fn = jax.jit(jax.shard_map(shard_fn, mesh=mesh,
                               in_specs=in_specs, out_specs=out_specs))

    shardings = tuple(NamedSharding(mesh, s) for s in in_specs)
    return mesh, fn, shardings


def kernel(**inputs):
    if "fn" not in _cache:
        _cache["mesh"], _cache["fn"], _cache["shardings"] = _build()
    fn, shardings = _cache["fn"], _cache["shardings"]

    order = ["static", "dynamic", "sbn_w", "sbn_b", "dbn_w", "dbn_b",
             "Ws", "bs", "Wd", "bd", "Wa", "ba", "Wp", "bp"]
    args = [jax.device_put(np.asarray(inputs[k]), sh)
            for k, sh in zip(order, shardings)]
    action, action_logp = fn(*args)
    return (np.asarray(jax.device_get(action)),
            np.asarray(jax.device_get(action_logp)))


# revision 5
# speedup vs baseline: 33.2633x; 1.3160x over previous
"""Data-parallel Trainium2 kernel for nn_Actor_89842125897993.

Shards the batch dim B=4096 across 8 NeuronCores (512 batches/core).
All params are replicated. The 20-step scan runs per-shard; the only
cross-core coupling is the training-mode BatchNorm statistics, which
are computed with explicit psum allreduces (6 floats per BN) so they
match the full-batch reference semantics exactly.

Self-contained: hardcodes all shapes from the problem spec.
"""
import numpy as np
import jax
import jax.numpy as jnp
from jax.sharding import Mesh, PartitionSpec as P, NamedSharding
from functools import partial

B, N, T = 4096, 16, 20
S, D = 3, 3
H, A, PP = 128, 8, 8
EPS = 1e-5
M = 8  # cores

_cache = {}


def _build():
    devs = jax.devices()[:M]
    mesh = Mesh(np.array(devs), ("x",))

    def bn_global(x, w, b):
        # x: [Bl, N, C]; global (full-batch) mean/var via psum, matching
        # the reference's two-pass biased variance.
        m = jax.lax.psum(jnp.sum(x, axis=(0, 1)), "x") / (B * N)
        v = jax.lax.psum(jnp.sum((x - m) ** 2, axis=(0, 1)), "x") / (B * N)
        return (x - m) * jax.lax.rsqrt(v + EPS) * w + b

    def shard_fn(static, dynamic, sbn_w, sbn_b, dbn_w, dbn_b,
                 Ws, bs, Wd, bd, Wa, ba, Wp, bp):
        dt = static.dtype

        # The network is linear up to the softmax/argmax, so fold
        # BN-affine + Ws/Wd + Wa/Wp into per-step [3,16] weights:
        #   out = static_t @ (s_scale*Wsx) + dyn_t @ (d_scale*Wdx) + const
        Wsx = Ws @ jnp.concatenate([Wa[:H], Wp[:H]], axis=1)        # [3,16]
        Wdx = Wd @ jnp.concatenate([Wa[H:], Wp[H:]], axis=1)        # [3,16]
        cx = (bs @ jnp.concatenate([Wa[:H], Wp[:H]], axis=1)
              + bd @ jnp.concatenate([Wa[H:], Wp[H:]], axis=1)
              + jnp.concatenate([ba, bp]))                          # [16]

        # static BN stats for all T steps, once (global over full batch)
        sm = jax.lax.psum(jnp.sum(static, axis=(0, 1)), "x") / (B * N)   # [3,T]
        sv = jax.lax.psum(jnp.sum((static - sm) ** 2, axis=(0, 1)), "x") / (B * N)
        s_scale = sbn_w[:, None] * jax.lax.rsqrt(sv + EPS)               # [3,T]
        s_shift = sbn_b[:, None] - sm * s_scale                          # [3,T]

        def step(dyn_t, xs):
            static_t, s_scale_t, s_shift_t = xs           # [Bl,N,3], [3], [3]
            # one collective per step: all-gather per-shard (mean, M2),
            # combine via Chan's parallel variance (exact, no cancellation)
            nloc = dyn_t.shape[0] * N
            m_i = jnp.mean(dyn_t, axis=(0, 1))                    # [3]
            M2_i = jnp.sum((dyn_t - m_i) ** 2, axis=(0, 1))       # [3]
            g = jax.lax.all_gather(jnp.concatenate([m_i, M2_i]), "x")  # [M,6]
            ms, M2s = g[:, :3], g[:, 3:]
            dm = jnp.mean(ms, axis=0)
            dv = (jnp.sum(M2s, axis=0)
                  + nloc * jnp.sum((ms - dm) ** 2, axis=0)) / (B * N)
            d_scale = dbn_w * jax.lax.rsqrt(dv + EPS)
            d_shift = dbn_b - dm * d_scale
            Ws_eff = s_scale_t[:, None] * Wsx             # [3,16]
            Wd_eff = d_scale[:, None] * Wdx               # [3,16]
            c_eff = cx + s_shift_t @ Wsx + d_shift @ Wdx  # [16]
            out16 = static_t @ Ws_eff + dyn_t @ Wd_eff + c_eff   # [Bl,N,16]
            actions = out16[:, :, :A]
            aparams = out16[:, :, A:]
            quant_probs = jax.nn.softmax(actions, axis=2)
            # argmax via single-operand reduces (variadic reduce unsupported):
            # first index attaining the max, matching jnp.argmax semantics.
            maxv = jnp.max(actions, axis=2, keepdims=True)
            idxs = jnp.arange(A, dtype=jnp.int32)
            eq = actions >= maxv
            ptr_quant = jnp.min(jnp.where(eq, idxs, A), axis=2)
            onehot = (idxs == ptr_quant[..., None]).astype(dt)
            bdw = jnp.sum(aparams * onehot, axis=2)
            bdw = jax.nn.softmax(bdw, axis=1)
            ptr_q8 = ptr_quant + 8
            d2 = dyn_t[:, :, 1] + 0.05 * static_t[:, :, 2]
            d2 = jnp.sqrt((d2 - 500.0) ** 2 + 100.0)
            rate = bdw * 10.0 * jnp.log2(1.0 + 1e7 * static_t[:, :, 0] / (d2 * d2))
            d0 = jnp.max(0.05 + ptr_q8.astype(dt) / 32.0 / rate, axis=1, keepdims=True)
            d1 = d0 * static_t[:, :, 2] + dyn_t[:, :, 1]
            new_dyn = jnp.stack([jnp.broadcast_to(d0, d2.shape), d1, d2], axis=2)
            wprobs = jnp.full(quant_probs.shape[:2] + (2,), 0.5, dtype=dt)
            ptr_sel = jnp.zeros(quant_probs.shape[:2], dtype=jnp.int32)
            return new_dyn, (ptr_sel, ptr_q8, bdw, wprobs, quant_probs, aparams)

        static_steps = jnp.moveaxis(static, 3, 0)
        dyn0 = dynamic[:, :, :, 0]
        _, (ptr_sel, ptr_q8, bdw, wprobs, quant_probs, aparams) = jax.lax.scan(
            step, dyn0, (static_steps, s_scale.T, s_shift.T))

        tl = lambda x: jnp.moveaxis(x, 0, -1)
        action = jnp.concatenate([
            tl(ptr_sel).astype(dt)[:, :, None, :],
            tl(ptr_q8).astype(dt)[:, :, None, :],
            tl(bdw)[:, :, None, :]], axis=2)
        action_logp = jnp.concatenate(
            [tl(wprobs), tl(quant_probs), tl(aparams)], axis=2)
        return action, action_logp

    shard_spec = P("x")
    rep = P()
    in_specs = (shard_spec, shard_spec) + (rep,) * 12
    out_specs = (shard_spec, shard_spec)

    fn = jax.jit(jax.shard_map(shard_fn, mesh=mesh,
                               in_specs=in_specs, out_specs=out_specs))

    shardings = tuple(NamedSharding(mesh, s) for s in in_specs)
    return mesh, fn, shardings


def kernel(**inputs):
    if "fn" not in _cache:
        _cache["mesh"], _cache["fn"], _cache["shardings"] = _build()
    fn, shardings = _cache["fn"], _cache["shardings"]

    order = ["static", "dynamic", "sbn_w", "sbn_b", "dbn_w", "dbn_b",
             "Ws", "bs", "Wd", "bd", "Wa", "ba", "Wp", "bp"]
    args = [jax.device_put(np.asarray(inputs[k]), sh)
            for k, sh in zip(order, shardings)]
    action, action_logp = fn(*args)
    return (np.asarray(jax.device_get(action)),
            np.asarray(jax.device_get(action_logp)))
